# revision 2
# baseline (speedup 1.0000x reference)
"""MoE routing kernel for Trainium2, 8 NeuronCores, expert-parallel (V1: bf16).

Strategy
--------
Host: gate (x @ Wg + bg), top-2 + softmax -> routing metadata only; all
expert-MLP compute, the all-to-all and the combine run on device.

Expert-parallel, bucket-padded layout: core c runs expert c. Its routed
tokens are laid out host-side in owner-bucket order with each bucket padded
to CAP slots (SR = 8*CAP columns total, pad columns zero). Because bucket
boundaries are then STATIC (o*CAP), the mm2 output tile IS the all-to-all
send buffer -- no permutation matmuls at all:

  mm1 (bf16): h^T tile [f 128, tok] = W1tile^T @ xT      (psum, fp32 acc)
  act: t1 = Gelu(gg*ps + b1g), t2 = Silu(gs*ps + b1s)    (direct HW tables;
       per-core gate data gg/gs selects which one is live, the other is 0)
       aT = t1 + t2  (bf16)
  mm2 (bf16): y tile [tok 128, d 512] = sum_f aT_f^T @ W2rows_f  (psum)
       send tile = y * wcol (per-token combine weight, pad slots 0) -> bf16
  AllToAll (bf16) send -> recv
  combine: dma_gather(recv, idx1) + dma_gather(recv, idx2); y_shard =
       g1 + g2 + b2  (each owned token's two expert contributions live at
       host-computed recv rows; int16 index data drives the gather)
"""

import numpy as np
import ml_dtypes

D_MODEL, D_FF, N_EXPERTS, TOP_K = 1024, 4096, 8, 2
B, S = 2, 2048
T = B * S
NCORES = 8
P = 128
SHARD = T // NCORES     # 512 tokens owned per core
FD = D_FF // P          # 32 f-tiles
KD = D_MODEL // P       # 8 k-tiles (d_model)
TG = SHARD // P         # 4 owned-token tiles
DH = 2                  # d_model halves (512-col matmul free dim)
MAX_CAP = 160           # SR = 8*CAP <= 1280 (SBUF residency bound)

_prog_cache = {}
_wprep_cache = {}

_bf16 = ml_dtypes.bfloat16


def _chunks(n, step=512):
    out = []
    o = 0
    while o < n:
        L = min(step, n - o)
        out.append((o, L))
        o += L
    return out


def _build_program(CAP):
    import concourse.tile as tile
    from concourse import bacc, mybir, library_config

    f32 = mybir.dt.float32
    bf16 = mybir.dt.bfloat16
    i16 = mybir.dt.int16
    SR = NCORES * CAP
    G = SR // P
    CH = _chunks(SR)

    nc = bacc.Bacc("TRN2", target_bir_lowering=False, debug=False,
                   num_devices=NCORES)

    xT = nc.dram_tensor("xT", [D_MODEL, SR], bf16, kind="ExternalInput").ap()
    W1r = nc.dram_tensor("W1r", [FD, P, KD * P], bf16, kind="ExternalInput").ap()
    W2n = nc.dram_tensor("W2n", [D_FF, D_MODEL], bf16, kind="ExternalInput").ap()
    acts = nc.dram_tensor("acts", [P, 2], f32, kind="ExternalInput").ap()
    b1g = nc.dram_tensor("b1g", [P, FD], f32, kind="ExternalInput").ap()
    b1s = nc.dram_tensor("b1s", [P, FD], f32, kind="ExternalInput").ap()
    b2bc = nc.dram_tensor("b2bc", [P, D_MODEL], f32, kind="ExternalInput").ap()
    wct = nc.dram_tensor("wct", [P, G], f32, kind="ExternalInput").ap()
    idx1 = nc.dram_tensor("idx1", [P, SHARD // 16], i16, kind="ExternalInput").ap()
    idx2 = nc.dram_tensor("idx2", [P, SHARD // 16], i16, kind="ExternalInput").ap()
    y_shard = nc.dram_tensor("y_shard", [SHARD, D_MODEL], bf16,
                             kind="ExternalOutput").ap()

    send_buf = nc.dram_tensor("send_buf", [SR, D_MODEL], bf16).ap()
    recv_buf = nc.dram_tensor("recv_buf", [SR, D_MODEL], bf16).ap()

    with tile.TileContext(nc) as tc:
        with (
            tc.tile_pool(name="xtp", bufs=1) as xtp,
            tc.tile_pool(name="atp", bufs=1) as atp,
            tc.tile_pool(name="w1p", bufs=3) as w1p,
            tc.tile_pool(name="w2p", bufs=1) as w2p,
            tc.tile_pool(name="smalls", bufs=1) as smalls,
            tc.tile_pool(name="actp", bufs=4) as actp,
            tc.tile_pool(name="sndp", bufs=4) as sndp,
            tc.tile_pool(name="cmb", bufs=1) as cmb,
            tc.tile_pool(name="psm1", bufs=4, space="PSUM") as psm1,
            tc.tile_pool(name="psm2", bufs=4, space="PSUM") as psm2,
        ):
            nc.gpsimd.load_library(library_config.mlp)

            actt = smalls.tile([P, 2], f32, tag="actt")
            nc.sync.dma_start(out=actt[:], in_=acts[:, :])
            b1gt = smalls.tile([P, FD], f32, tag="b1gt")
            nc.sync.dma_start(out=b1gt[:], in_=b1g[:, :])
            b1st = smalls.tile([P, FD], f32, tag="b1st")
            nc.sync.dma_start(out=b1st[:], in_=b1s[:, :])
            b2t = smalls.tile([P, D_MODEL], f32, tag="b2t")
            nc.sync.dma_start(out=b2t[:], in_=b2bc[:, :])
            wctt = smalls.tile([P, G], f32, tag="wctt")
            nc.sync.dma_start(out=wctt[:], in_=wct[:, :])
            ix1 = smalls.tile([P, SHARD // 16], i16, tag="ix1")
            nc.sync.dma_start(out=ix1[:], in_=idx1[:, :])
            ix2 = smalls.tile([P, SHARD // 16], i16, tag="ix2")
            nc.sync.dma_start(out=ix2[:], in_=idx2[:, :])

            xts = []
            for k in range(KD):
                xt = xtp.tile([P, SR], bf16, tag=f"xt{k}", name=f"xt{k}")
                nc.sync.dma_start(out=xt[:], in_=xT[k * P:(k + 1) * P, :])
                xts.append(xt)

            aT = []
            for f in range(FD):
                a = atp.tile([P, SR], bf16, tag=f"aT{f}", name=f"aT{f}")
                aT.append(a)

            # ---- mm1 + activation -> aT
            for f in range(FD):
                w1f = w1p.tile([P, KD * P], bf16, tag="w1f")
                nc.sync.dma_start(out=w1f[:], in_=W1r[f])
                for (o, L) in CH:
                    ps = psm1.tile([P, 512], mybir.dt.float32, tag="ps1")
                    for k in range(KD):
                        nc.tensor.matmul(ps[:, :L],
                                         lhsT=w1f[:, k * P:(k + 1) * P],
                                         rhs=xts[k][:, o:o + L],
                                         start=(k == 0), stop=(k == KD - 1))
                    t1 = actp.tile([P, 512], bf16, tag="t1")
                    t2 = actp.tile([P, 512], bf16, tag="t2")
                    nc.scalar.activation(
                        t1[:, :L], ps[:, :L],
                        mybir.ActivationFunctionType.Gelu,
                        bias=b1gt[:, f:f + 1], scale=actt[:, 0:1])
                    nc.scalar.activation(
                        t2[:, :L], ps[:, :L],
                        mybir.ActivationFunctionType.Silu,
                        bias=b1st[:, f:f + 1], scale=actt[:, 1:2])
                    nc.vector.tensor_add(aT[f][:, o:o + L], t1[:, :L], t2[:, :L])

            # ---- mm2 (token-major) + combine-weight scale -> send_buf
            for dh in range(DH):
                w2ts = []
                for f in range(FD):
                    w2t = w2p.tile([P, 512], bf16, tag=f"w2_{f}", name=f"w2_{f}_{dh}")
                    nc.sync.dma_start(
                        out=w2t[:],
                        in_=W2n[f * P:(f + 1) * P, dh * 512:(dh + 1) * 512])
                    w2ts.append(w2t)
                for g in range(G):
                    ps = psm2.tile([P, 512], mybir.dt.float32, tag="ps2")
                    for f in range(FD):
                        nc.tensor.matmul(ps[:],
                                         lhsT=aT[f][:, g * P:(g + 1) * P],
                                         rhs=w2ts[f][:],
                                         start=(f == 0), stop=(f == FD - 1))
                    snd = sndp.tile([P, 512], bf16, tag="snd")
                    nc.vector.tensor_scalar_mul(snd[:], ps[:], wctt[:, g:g + 1])
                    nc.sync.dma_start(
                        out=send_buf[g * P:(g + 1) * P, dh * 512:(dh + 1) * 512],
                        in_=snd[:])

            nc.gpsimd.collective_compute(
                "AllToAll",
                mybir.AluOpType.bypass,
                replica_groups=[list(range(NCORES))],
                ins=[send_buf[:, :]],
                outs=[recv_buf[:, :]],
            )

            # ---- combine: gather each owned token's two contribution rows
            g1 = cmb.tile([P, TG, D_MODEL], bf16, tag="g1", name="g1")
            nc.gpsimd.dma_gather(
                out_ap=g1[:, :, :], in_ap=recv_buf[:, :], idxs_ap=ix1[:],
                num_idxs=SHARD, num_idxs_reg=SHARD, elem_size=D_MODEL)
            g2 = cmb.tile([P, TG, D_MODEL], bf16, tag="g2", name="g2")
            nc.gpsimd.dma_gather(
                out_ap=g2[:, :, :], in_ap=recv_buf[:, :], idxs_ap=ix2[:],
                num_idxs=SHARD, num_idxs_reg=SHARD, elem_size=D_MODEL)
            nc.vector.tensor_add(g1[:, :, :], g1[:, :, :], g2[:, :, :])
            for tg in range(TG):
                nc.vector.tensor_add(g1[:, tg, :], g1[:, tg, :], b2t[:])
                nc.sync.dma_start(out=y_shard[tg * P:(tg + 1) * P, :],
                                  in_=g1[:, tg, :])

    nc.compile()
    return nc


def _route(x_flat, Wg, bg):
    logits = x_flat.astype(np.float32) @ Wg.astype(np.float32) + bg
    order = np.argsort(-logits, axis=1, kind="stable")
    i1, i2 = order[:, 0], order[:, 1]
    s1 = np.take_along_axis(logits, i1[:, None], 1)[:, 0]
    s2 = np.take_along_axis(logits, i2[:, None], 1)[:, 0]
    e = np.exp((s2 - s1).astype(np.float32))
    w1 = 1.0 / (1.0 + e)
    w2 = e * w1
    return i1, i2, w1.astype(np.float32), w2.astype(np.float32)


def _prep_weights(W1, W2):
    key = (id(W1), id(W2))
    hit = _wprep_cache.get(key)
    if hit is not None:
        return hit
    W1 = np.asarray(W1, np.float32)
    # W1r[e, f, p, k*128+q] = W1[e, k*128+p, f*128+q]
    W1r = np.ascontiguousarray(
        W1.reshape(N_EXPERTS, KD, P, FD, P).transpose(0, 3, 2, 1, 4)
        .reshape(N_EXPERTS, FD, P, KD * P).astype(_bf16))
    W2b = np.ascontiguousarray(np.asarray(W2, np.float32).astype(_bf16))
    _wprep_cache.clear()
    _wprep_cache[key] = (W1r, W2b)
    return W1r, W2b


def _prepare(x, W1, b1, W2, b2, Wg, bg):
    x = np.asarray(x, np.float32)
    b1 = np.asarray(b1, np.float32)
    b2 = np.asarray(b2, np.float32)
    x_flat = np.ascontiguousarray(x.reshape(T, D_MODEL))
    i1, i2, w1, w2 = _route(x_flat, np.asarray(Wg, np.float32),
                            np.asarray(bg, np.float32))
    W1r, W2b = _prep_weights(W1, W2)

    jobs = {}  # expert -> (ids ascending = sorted by owner, wts)
    for e in range(N_EXPERTS):
        sel = (i1 == e) | (i2 == e)
        ids = np.nonzero(sel)[0]
        wts = np.where(i1[ids] == e, w1[ids], w2[ids]).astype(np.float32)
        jobs[e] = (ids, wts)
    return x_flat, jobs, (W1r, W2b, b1, b2)


def _wrap_idx(r):
    """[n] int -> [128, n/16] int16 (wrapped by 16, replicated 8x)."""
    n = len(r)
    w = np.zeros((16, n // 16), np.int16)
    w[np.arange(n) % 16, np.arange(n) // 16] = r
    return np.tile(w, (8, 1))


def _pass_maps(x_flat, jobs, consts, first_pass=True, strict=False):
    W1r, W2b, b1, b2 = consts

    bucket_count = np.zeros((NCORES, NCORES), np.int64)
    for e in range(NCORES):
        ids, _ = jobs[e]
        own = ids // SHARD
        for o in range(NCORES):
            bucket_count[e, o] += (own == o).sum()
    CAP = max(16, int(-(-bucket_count.max() // 16) * 16))
    assert CAP <= MAX_CAP
    SR = NCORES * CAP
    G = SR // P

    # recv row (on the owner) of each token contribution
    src_rows = np.full((T, 2), -1, np.int64)
    slot_of = {}
    for e in range(NCORES):
        ids, _ = jobs[e]
        own = ids // SHARD
        ks = np.empty(len(ids), np.int64)
        fill = np.zeros(NCORES, np.int64)
        for o in range(NCORES):
            m = own == o
            n = int(m.sum())
            ks[m] = fill[o] + np.arange(n)
            fill[o] += n
        slot_of[e] = own * CAP + ks
        rows_recv = e * CAP + ks
        which = (src_rows[ids, 0] >= 0).astype(np.int64)
        src_rows[ids, which] = rows_recv
    if strict:
        assert (src_rows >= 0).all()

    in_maps = []
    for c in range(NCORES):
        e = c
        ids, wts = jobs[e]
        xTc = np.zeros((D_MODEL, SR), _bf16)
        wcol = np.zeros(SR, np.float32)
        if len(ids):
            slots = slot_of[e]
            xTc[:, slots] = x_flat[ids].T.astype(_bf16)
            wcol[slots] = wts
        even = (e % 2 == 0)
        b1_cols = np.ascontiguousarray(b1[e].reshape(FD, P).T)  # [P, FD]
        actsel = np.zeros((P, 2), np.float32)
        actsel[:, 0] = 1.0 if even else 0.0
        actsel[:, 1] = 0.0 if even else 1.0
        b1g = b1_cols if even else np.zeros((P, FD), np.float32)
        b1s = np.zeros((P, FD), np.float32) if even else b1_cols
        b2v = b2[e] if first_pass else np.zeros(D_MODEL, np.float32)
        tok0 = c * SHARD
        r1 = src_rows[tok0:tok0 + SHARD, 0]
        r2 = src_rows[tok0:tok0 + SHARD, 1]
        # tokens with a single contribution in this pass: duplicate with
        # zero... cannot happen with strict top-2; under npass splitting a
        # token's two contributions may land in different passes -> the
        # missing one must contribute zero: point it at a pad row (wcol 0
        # there makes the gathered row zero only if the slot was never
        # filled; pad rows of send_buf hold zero because their xT columns
        # are zero and wcol is zero -> y*0). Use row SR-1's pad only if
        # that bucket is not full; safest is a dedicated zero row: slot
        # CAP-1 of bucket 7 may be in use, so instead reuse the token's
        # present contribution with weight already folded: gather the same
        # row twice would double it. Simplest correct fallback: point the
        # missing contribution at any slot with wcol == 0 (a pad slot).
        if not strict:
            pad_slot = _find_pad_row(bucket_count, CAP)
            r1 = np.where(r1 < 0, pad_slot, r1)
            r2 = np.where(r2 < 0, pad_slot, r2)
        in_maps.append({
            "xT": xTc,
            "W1r": W1r[e], "W2n": W2b[e],
            "acts": actsel, "b1g": np.ascontiguousarray(b1g),
            "b1s": np.ascontiguousarray(b1s),
            "b2bc": np.ascontiguousarray(
                np.broadcast_to(b2v, (P, D_MODEL)).astype(np.float32)),
            "wct": np.ascontiguousarray(wcol.reshape(G, P).T),
            "idx1": _wrap_idx(r1), "idx2": _wrap_idx(r2),
        })
    return (CAP,), in_maps


def _find_pad_row(bucket_count, CAP):
    """Recv row index that is guaranteed to be a zero pad slot on every core.

    Receiver side: recv chunk e holds sender e's bucket; slot k >= count
    (e, o=receiver) is pad. Need a (e, k) pad for ALL receivers: pick e, k
    with k >= max_o bucket_count[e, o]."""
    per_e_max = bucket_count.max(axis=1)
    e = int(per_e_max.argmin())
    k = int(per_e_max[e])
    assert k < CAP, "no universal pad slot (all buckets full)"
    return e * CAP + k


def make_in_maps(x, W1, b1, W2, b2, Wg, bg):
    x_flat, jobs, consts = _prepare(x, W1, b1, W2, b2, Wg, bg)
    return _pass_maps(x_flat, jobs, consts, strict=True)


def get_program(key):
    if key not in _prog_cache:
        _prog_cache[key] = _build_program(*key)
    return _prog_cache[key]


def kernel(x, W1, b1, W2, b2, Wg, bg):
    from concourse.bass_utils import run_bass_kernel_spmd

    x_flat, jobs, consts = _prepare(x, W1, b1, W2, b2, Wg, bg)
    maxbucket = 0
    for e in range(N_EXPERTS):
        own = jobs[e][0] // SHARD
        if len(own):
            maxbucket = max(maxbucket, int(np.bincount(own).max()))
    npass = max(1, -(-maxbucket // MAX_CAP))
    out = None
    for p in range(npass):
        jobs_p = {e: (ids[p::npass], wts[p::npass])
                  for e, (ids, wts) in jobs.items()}
        key, in_maps = _pass_maps(x_flat, jobs_p, consts,
                                  first_pass=(p == 0), strict=(npass == 1))
        nc = get_program(key)
        res = run_bass_kernel_spmd(nc, in_maps, list(range(NCORES)))
        full = np.concatenate(
            [res.results[c]["y_shard"].astype(np.float32)
             for c in range(NCORES)], axis=0)
        out = full if out is None else out + full
    return np.ascontiguousarray(out.reshape(B, S, D_MODEL))


# revision 9
# speedup vs baseline: 1.9496x; 1.9496x over previous
"""MoE routing kernel for Trainium2, 8 NeuronCores, expert-parallel (V1: bf16).

Strategy
--------
Host: gate (x @ Wg + bg), top-2 + softmax -> routing metadata only; all
expert-MLP compute, the all-to-all and the combine run on device.

Expert-parallel, bucket-padded layout: core c runs expert c. Its routed
tokens are laid out host-side in owner-bucket order with each bucket padded
to CAP slots (SR = 8*CAP columns total, pad columns zero). Because bucket
boundaries are then STATIC (o*CAP), the mm2 output tile IS the all-to-all
send buffer -- no permutation matmuls at all:

  mm1 (bf16): h^T tile [f 128, tok] = W1tile^T @ xT      (psum, fp32 acc)
  act (Erf + Sigmoid share one ACT table set -> no table reloads; function
       choice is DATA via per-core scale/bias):
       t1 = Erf(s1*ps + bg)      even: s1=1/sqrt2, bg=b1/sqrt2   odd: 0 -> 0
       t2 = Sigmoid(s2*ps + bs)  even: 0 -> 0.5 (the gelu const) odd: 1, b1
       v = 0.5*t1 + t2 ; aT = (ps + b1) * v   (bf16)
  mm2 (bf16): y tile [tok 128, d 512] = sum_f aT_f^T @ W2rows_f  (psum)
       send tile = y * wcol (per-token combine weight, pad slots 0) -> bf16
  AllToAll (bf16) send -> recv, split into two d_model halves so the first
       half's collective + combine overlap the second half of mm2
  combine: dma_gather(recv, idx1) + dma_gather(recv, idx2); y_shard =
       g1 + g2 + b2  (each owned token's two expert contributions live at
       host-computed recv rows; int16 index data drives the gather)
"""

import numpy as np
import ml_dtypes

D_MODEL, D_FF, N_EXPERTS, TOP_K = 1024, 4096, 8, 2
B, S = 2, 2048
T = B * S
NCORES = 8
P = 128
SHARD = T // NCORES     # 512 tokens owned per core
FD = D_FF // P          # 32 f-tiles
KD = D_MODEL // P       # 8 k-tiles (d_model)
TG = SHARD // P         # 4 owned-token tiles
DH = 2                  # d_model halves (512-col matmul free dim)
MAX_CAP = 160           # SR = 8*CAP <= 1280 (SBUF residency bound)

_prog_cache = {}
_wprep_cache = {}

_bf16 = ml_dtypes.bfloat16


def _chunks(n, step=512):
    out = []
    o = 0
    while o < n:
        L = min(step, n - o)
        out.append((o, L))
        o += L
    return out


def _build_program(CAP):
    import concourse.tile as tile
    from concourse import bacc, mybir, library_config

    f32 = mybir.dt.float32
    bf16 = mybir.dt.bfloat16
    i16 = mybir.dt.int16
    SR = NCORES * CAP
    G = SR // P
    CH = _chunks(SR)

    nc = bacc.Bacc("TRN2", target_bir_lowering=False, debug=False,
                   num_devices=NCORES)

    xT = nc.dram_tensor("xT", [D_MODEL, SR], bf16, kind="ExternalInput").ap()
    W1r = nc.dram_tensor("W1r", [FD, P, KD * P], bf16, kind="ExternalInput").ap()
    W2n = nc.dram_tensor("W2n", [D_FF, D_MODEL], bf16, kind="ExternalInput").ap()
    acts = nc.dram_tensor("acts", [P, 2], f32, kind="ExternalInput").ap()
    b1g = nc.dram_tensor("b1g", [P, FD], f32, kind="ExternalInput").ap()
    b1s = nc.dram_tensor("b1s", [P, FD], f32, kind="ExternalInput").ap()
    b1a = nc.dram_tensor("b1a", [P, FD], f32, kind="ExternalInput").ap()
    b2bc = nc.dram_tensor("b2bc", [P, D_MODEL], f32, kind="ExternalInput").ap()
    wct = nc.dram_tensor("wct", [P, G], f32, kind="ExternalInput").ap()
    idx1 = nc.dram_tensor("idx1", [P, SHARD // 16], i16, kind="ExternalInput").ap()
    idx2 = nc.dram_tensor("idx2", [P, SHARD // 16], i16, kind="ExternalInput").ap()
    y_shard = nc.dram_tensor("y_shard", [SHARD, D_MODEL], bf16,
                             kind="ExternalOutput").ap()

    send_h = [nc.dram_tensor(f"send_h{dh}", [SR, 512], bf16).ap()
              for dh in range(DH)]
    recv_h = [nc.dram_tensor(f"recv_h{dh}", [SR, 512], bf16).ap()
              for dh in range(DH)]

    with tile.TileContext(nc) as tc:
        with (
            tc.tile_pool(name="xtp", bufs=1) as xtp,
            tc.tile_pool(name="atp", bufs=1) as atp,
            tc.tile_pool(name="w1p", bufs=3) as w1p,
            tc.tile_pool(name="w2p", bufs=1) as w2p,
            tc.tile_pool(name="smalls", bufs=1) as smalls,
            tc.tile_pool(name="actp", bufs=4) as actp,
            tc.tile_pool(name="sndp", bufs=4) as sndp,
            tc.tile_pool(name="cmb", bufs=1) as cmb,
            tc.tile_pool(name="psm1", bufs=4, space="PSUM") as psm1,
            tc.tile_pool(name="psm2", bufs=4, space="PSUM") as psm2,
        ):
            nc.gpsimd.load_library(library_config.mlp)

            actt = smalls.tile([P, 2], f32, tag="actt")
            nc.sync.dma_start(out=actt[:], in_=acts[:, :])
            b1gt = smalls.tile([P, FD], f32, tag="b1gt")
            nc.sync.dma_start(out=b1gt[:], in_=b1g[:, :])
            b1st = smalls.tile([P, FD], f32, tag="b1st")
            nc.sync.dma_start(out=b1st[:], in_=b1s[:, :])
            b1at = smalls.tile([P, FD], f32, tag="b1at")
            nc.sync.dma_start(out=b1at[:], in_=b1a[:, :])
            b2t = smalls.tile([P, D_MODEL], f32, tag="b2t")
            nc.sync.dma_start(out=b2t[:], in_=b2bc[:, :])
            wctt = smalls.tile([P, G], f32, tag="wctt")
            nc.sync.dma_start(out=wctt[:], in_=wct[:, :])
            ix1 = smalls.tile([P, SHARD // 16], i16, tag="ix1")
            nc.sync.dma_start(out=ix1[:], in_=idx1[:, :])
            ix2 = smalls.tile([P, SHARD // 16], i16, tag="ix2")
            nc.sync.dma_start(out=ix2[:], in_=idx2[:, :])

            xts = []
            for k in range(KD):
                xt = xtp.tile([P, SR], bf16, tag=f"xt{k}", name=f"xt{k}")
                nc.sync.dma_start(out=xt[:], in_=xT[k * P:(k + 1) * P, :])
                xts.append(xt)

            aT = []
            for f in range(FD):
                a = atp.tile([P, SR], bf16, tag=f"aT{f}", name=f"aT{f}")
                aT.append(a)

            # ---- mm1 + activation -> aT
            for f in range(FD):
                w1f = w1p.tile([P, KD * P], bf16, tag="w1f")
                nc.sync.dma_start(out=w1f[:], in_=W1r[f])
                for (o, L) in CH:
                    ps = psm1.tile([P, 512], mybir.dt.float32, tag="ps1")
                    for k in range(KD):
                        nc.tensor.matmul(ps[:, :L],
                                         lhsT=w1f[:, k * P:(k + 1) * P],
                                         rhs=xts[k][:, o:o + L],
                                         start=(k == 0), stop=(k == KD - 1))
                    t1 = actp.tile([P, 512], bf16, tag="t1")
                    t2 = actp.tile([P, 512], bf16, tag="t2")
                    nc.scalar.activation(
                        t1[:, :L], ps[:, :L],
                        mybir.ActivationFunctionType.Erf,
                        bias=b1gt[:, f:f + 1], scale=actt[:, 0:1])
                    nc.scalar.activation(
                        t2[:, :L], ps[:, :L],
                        mybir.ActivationFunctionType.Sigmoid,
                        bias=b1st[:, f:f + 1], scale=actt[:, 1:2])
                    # v = 0.5*t1 + t2 ; aT = (ps + b1) * v
                    nc.vector.scalar_tensor_tensor(
                        out=t1[:, :L], in0=t1[:, :L], scalar=0.5,
                        in1=t2[:, :L],
                        op0=mybir.AluOpType.mult, op1=mybir.AluOpType.add)
                    nc.vector.scalar_tensor_tensor(
                        out=aT[f][:, o:o + L], in0=ps[:, :L],
                        scalar=b1at[:, f:f + 1], in1=t1[:, :L],
                        op0=mybir.AluOpType.add, op1=mybir.AluOpType.mult)

            # ---- mm2 (token-major) + combine-weight scale -> send halves;
            # the dh=0 collective + combine overlap the dh=1 matmuls.
            for dh in range(DH):
                w2ts = []
                for f in range(FD):
                    w2t = w2p.tile([P, 512], bf16, tag=f"w2_{f}", name=f"w2_{f}_{dh}")
                    nc.sync.dma_start(
                        out=w2t[:],
                        in_=W2n[f * P:(f + 1) * P, dh * 512:(dh + 1) * 512])
                    w2ts.append(w2t)
                for g in range(G):
                    ps = psm2.tile([P, 512], mybir.dt.float32, tag="ps2")
                    for f in range(FD):
                        nc.tensor.matmul(ps[:],
                                         lhsT=aT[f][:, g * P:(g + 1) * P],
                                         rhs=w2ts[f][:],
                                         start=(f == 0), stop=(f == FD - 1))
                    snd = sndp.tile([P, 512], bf16, tag="snd")
                    nc.vector.tensor_scalar_mul(snd[:], ps[:], wctt[:, g:g + 1])
                    nc.sync.dma_start(out=send_h[dh][g * P:(g + 1) * P, :],
                                      in_=snd[:])

                nc.gpsimd.collective_compute(
                    "AllToAll",
                    mybir.AluOpType.bypass,
                    replica_groups=[list(range(NCORES))],
                    ins=[send_h[dh][:, :]],
                    outs=[recv_h[dh][:, :]],
                )

                # combine: gather each owned token's two contribution rows
                g1 = cmb.tile([P, TG, 512], bf16, tag=f"g1_{dh}", name=f"g1_{dh}")
                nc.gpsimd.dma_gather(
                    out_ap=g1[:, :, :], in_ap=recv_h[dh][:, :], idxs_ap=ix1[:],
                    num_idxs=SHARD, num_idxs_reg=SHARD, elem_size=512)
                g2 = cmb.tile([P, TG, 512], bf16, tag=f"g2_{dh}", name=f"g2_{dh}")
                nc.gpsimd.dma_gather(
                    out_ap=g2[:, :, :], in_ap=recv_h[dh][:, :], idxs_ap=ix2[:],
                    num_idxs=SHARD, num_idxs_reg=SHARD, elem_size=512)
                nc.vector.tensor_add(g1[:, :, :], g1[:, :, :], g2[:, :, :])
                for tg in range(TG):
                    nc.vector.tensor_add(g1[:, tg, :], g1[:, tg, :],
                                         b2t[:, dh * 512:(dh + 1) * 512])
                    nc.sync.dma_start(
                        out=y_shard[tg * P:(tg + 1) * P,
                                    dh * 512:(dh + 1) * 512],
                        in_=g1[:, tg, :])

    nc.compile()
    return nc


def _route(x_flat, Wg, bg):
    logits = x_flat.astype(np.float32) @ Wg.astype(np.float32) + bg
    order = np.argsort(-logits, axis=1, kind="stable")
    i1, i2 = order[:, 0], order[:, 1]
    s1 = np.take_along_axis(logits, i1[:, None], 1)[:, 0]
    s2 = np.take_along_axis(logits, i2[:, None], 1)[:, 0]
    e = np.exp((s2 - s1).astype(np.float32))
    w1 = 1.0 / (1.0 + e)
    w2 = e * w1
    return i1, i2, w1.astype(np.float32), w2.astype(np.float32)


def _prep_weights(W1, W2):
    key = (id(W1), id(W2))
    hit = _wprep_cache.get(key)
    if hit is not None:
        return hit
    W1 = np.asarray(W1, np.float32)
    # W1r[e, f, p, k*128+q] = W1[e, k*128+p, f*128+q]
    W1r = np.ascontiguousarray(
        W1.reshape(N_EXPERTS, KD, P, FD, P).transpose(0, 3, 2, 1, 4)
        .reshape(N_EXPERTS, FD, P, KD * P).astype(_bf16))
    W2b = np.ascontiguousarray(np.asarray(W2, np.float32).astype(_bf16))
    _wprep_cache.clear()
    _wprep_cache[key] = (W1r, W2b)
    return W1r, W2b


def _prepare(x, W1, b1, W2, b2, Wg, bg):
    x = np.asarray(x, np.float32)
    b1 = np.asarray(b1, np.float32)
    b2 = np.asarray(b2, np.float32)
    x_flat = np.ascontiguousarray(x.reshape(T, D_MODEL))
    i1, i2, w1, w2 = _route(x_flat, np.asarray(Wg, np.float32),
                            np.asarray(bg, np.float32))
    W1r, W2b = _prep_weights(W1, W2)

    jobs = {}  # expert -> (ids ascending = sorted by owner, wts)
    for e in range(N_EXPERTS):
        sel = (i1 == e) | (i2 == e)
        ids = np.nonzero(sel)[0]
        wts = np.where(i1[ids] == e, w1[ids], w2[ids]).astype(np.float32)
        jobs[e] = (ids, wts)
    return x_flat, jobs, (W1r, W2b, b1, b2)


def _wrap_idx(r):
    """[n] int -> [128, n/16] int16 (wrapped by 16, replicated 8x)."""
    n = len(r)
    w = np.zeros((16, n // 16), np.int16)
    w[np.arange(n) % 16, np.arange(n) // 16] = r
    return np.tile(w, (8, 1))


def _pass_maps(x_flat, jobs, consts, first_pass=True, strict=False):
    W1r, W2b, b1, b2 = consts

    bucket_count = np.zeros((NCORES, NCORES), np.int64)
    for e in range(NCORES):
        ids, _ = jobs[e]
        own = ids // SHARD
        for o in range(NCORES):
            bucket_count[e, o] += (own == o).sum()
    CAP = max(16, int(-(-bucket_count.max() // 16) * 16))
    assert CAP <= MAX_CAP
    SR = NCORES * CAP
    G = SR // P

    # recv row (on the owner) of each token contribution
    src_rows = np.full((T, 2), -1, np.int64)
    slot_of = {}
    for e in range(NCORES):
        ids, _ = jobs[e]
        own = ids // SHARD
        ks = np.empty(len(ids), np.int64)
        fill = np.zeros(NCORES, np.int64)
        for o in range(NCORES):
            m = own == o
            n = int(m.sum())
            ks[m] = fill[o] + np.arange(n)
            fill[o] += n
        slot_of[e] = own * CAP + ks
        rows_recv = e * CAP + ks
        which = (src_rows[ids, 0] >= 0).astype(np.int64)
        src_rows[ids, which] = rows_recv
    if strict:
        assert (src_rows >= 0).all()

    in_maps = []
    for c in range(NCORES):
        e = c
        ids, wts = jobs[e]
        xTc = np.zeros((D_MODEL, SR), _bf16)
        wcol = np.zeros(SR, np.float32)
        if len(ids):
            slots = slot_of[e]
            xTc[:, slots] = x_flat[ids].T.astype(_bf16)
            wcol[slots] = wts
        even = (e % 2 == 0)
        sq2 = np.float32(1.0 / np.sqrt(2.0))
        b1_cols = np.ascontiguousarray(b1[e].reshape(FD, P).T)  # [P, FD]
        actsel = np.zeros((P, 2), np.float32)
        actsel[:, 0] = sq2 if even else 0.0
        actsel[:, 1] = 0.0 if even else 1.0
        b1g = b1_cols * sq2 if even else np.zeros((P, FD), np.float32)
        b1s = np.zeros((P, FD), np.float32) if even else b1_cols
        b2v = b2[e] if first_pass else np.zeros(D_MODEL, np.float32)
        tok0 = c * SHARD
        r1 = src_rows[tok0:tok0 + SHARD, 0]
        r2 = src_rows[tok0:tok0 + SHARD, 1]
        # tokens with a single contribution in this pass: duplicate with
        # zero... cannot happen with strict top-2; under npass splitting a
        # token's two contributions may land in different passes -> the
        # missing one must contribute zero: point it at a pad row (wcol 0
        # there makes the gathered row zero only if the slot was never
        # filled; pad rows of send_buf hold zero because their xT columns
        # are zero and wcol is zero -> y*0). Use row SR-1's pad only if
        # that bucket is not full; safest is a dedicated zero row: slot
        # CAP-1 of bucket 7 may be in use, so instead reuse the token's
        # present contribution with weight already folded: gather the same
        # row twice would double it. Simplest correct fallback: point the
        # missing contribution at any slot with wcol == 0 (a pad slot).
        if not strict:
            pad_slot = _find_pad_row(bucket_count, CAP)
            r1 = np.where(r1 < 0, pad_slot, r1)
            r2 = np.where(r2 < 0, pad_slot, r2)
        in_maps.append({
            "xT": xTc,
            "W1r": W1r[e], "W2n": W2b[e],
            "acts": actsel, "b1g": np.ascontiguousarray(b1g),
            "b1s": np.ascontiguousarray(b1s), "b1a": b1_cols,
            "b2bc": np.ascontiguousarray(
                np.broadcast_to(b2v, (P, D_MODEL)).astype(np.float32)),
            "wct": np.ascontiguousarray(wcol.reshape(G, P).T),
            "idx1": _wrap_idx(r1), "idx2": _wrap_idx(r2),
        })
    return (CAP,), in_maps


def _find_pad_row(bucket_count, CAP):
    """Recv row index that is guaranteed to be a zero pad slot on every core.

    Receiver side: recv chunk e holds sender e's bucket; slot k >= count
    (e, o=receiver) is pad. Need a (e, k) pad for ALL receivers: pick e, k
    with k >= max_o bucket_count[e, o]."""
    per_e_max = bucket_count.max(axis=1)
    e = int(per_e_max.argmin())
    k = int(per_e_max[e])
    assert k < CAP, "no universal pad slot (all buckets full)"
    return e * CAP + k


def make_in_maps(x, W1, b1, W2, b2, Wg, bg):
    x_flat, jobs, consts = _prepare(x, W1, b1, W2, b2, Wg, bg)
    return _pass_maps(x_flat, jobs, consts, strict=True)


def get_program(key):
    if key not in _prog_cache:
        _prog_cache[key] = _build_program(*key)
    return _prog_cache[key]


def kernel(x, W1, b1, W2, b2, Wg, bg):
    from concourse.bass_utils import run_bass_kernel_spmd

    x_flat, jobs, consts = _prepare(x, W1, b1, W2, b2, Wg, bg)
    maxbucket = 0
    for e in range(N_EXPERTS):
        own = jobs[e][0] // SHARD
        if len(own):
            maxbucket = max(maxbucket, int(np.bincount(own).max()))
    npass = max(1, -(-maxbucket // MAX_CAP))
    out = None
    for p in range(npass):
        jobs_p = {e: (ids[p::npass], wts[p::npass])
                  for e, (ids, wts) in jobs.items()}
        key, in_maps = _pass_maps(x_flat, jobs_p, consts,
                                  first_pass=(p == 0), strict=(npass == 1))
        nc = get_program(key)
        res = run_bass_kernel_spmd(nc, in_maps, list(range(NCORES)))
        full = np.concatenate(
            [res.results[c]["y_shard"].astype(np.float32)
             for c in range(NCORES)], axis=0)
        out = full if out is None else out + full
    return np.ascontiguousarray(out.reshape(B, S, D_MODEL))


# revision 14
# speedup vs baseline: 2.3661x; 1.2136x over previous
"""MoE routing kernel for Trainium2, 8 NeuronCores, expert-parallel.

V2: 3-stream error-compensated fp8 DoubleRow matmuls.

Host: gate (x @ Wg + bg), top-2 + softmax -> routing metadata only; all
expert-MLP compute, the all-to-all and the combine run on device.

Expert-parallel, bucket-padded layout: core c runs expert c. Its routed
tokens are laid out host-side in owner-bucket order with each bucket padded
to CAP slots (SR = 8*CAP columns, pad columns zero). Bucket boundaries are
then STATIC (o*CAP), so the mm2 output tile IS the all-to-all send buffer:
no permutation matmuls.

Matmuls run as fp8 DoubleRow (256-row contraction per instruction at 0.5
cyc/col -- 4x the bf16/fp32r rate) with hi/lo error compensation:
  A = Ah + Al (Ah = e4m3(A), Al = e5m2 residual -- e5m2's exponent range
  holds the small residuals exactly where e4m3 subnormals would not)
  A@B ~= Ah@Bh + Al@Bh + Ah@Bl   (the dropped Al@Bl term is ~0.1% noise)
3 DR streams per 256-rows = 1.5 cyc/col vs bf16's 2.0, with bf16-level
accuracy (verified 3.8e-3 rel err vs 4.1e-3 all-bf16).

W1/W2 are pre-scaled by 32 so their e4m3 mantissas sit in the normal range;
psum1 = 32*h, descaled inside the ACT affine args; aT is built as 32*act
(e4m3-safe: |32*act| < 240) and psum2 = 1024*y, descaled via wct.

  mm1: psum[f 128, tok] = sum_k2 {W1h (x) xh + W1h (x) xl + W1l (x) xh}
  act (Erf + Sigmoid share one ACT table set -> no table reloads; function
       choice is DATA via per-core scale/bias):
       t1 = Erf(s1*ps + bg)      even: s1=1/(32*sqrt2), bg=b1/sqrt2; odd: 0
       t2 = Sigmoid(s2*ps + bs)  even: 0 -> 0.5 (the gelu const); odd: 1/32
       v = 0.5*t1 + t2; temp = (ps + 32*b1)*v = 32*aT (bf16)
       ah = e4m3(temp) [DVE]; al = e5m2(temp - ah) [Pool]
  mm2: psum[tok 128, d 512] = sum_p2 {ah (x) W2h + al (x) W2h + ah (x) W2l}
       send tile = psum * (wcol/1024) -> bf16
  AllToAll (bf16) send -> recv, split into two d_model halves so the first
       half's collective + combine overlap the second half of mm2
  combine: dma_gather(recv, idx1) + dma_gather(recv, idx2); y_shard =
       g1 + g2 + b2 (int16 index data drives the gathers)
"""

import numpy as np
import ml_dtypes

D_MODEL, D_FF, N_EXPERTS, TOP_K = 1024, 4096, 8, 2
B, S = 2, 2048
T = B * S
NCORES = 8
P = 128
SHARD = T // NCORES     # 512 tokens owned per core
FD = D_FF // P          # 32 f-tiles
FD2 = FD // 2           # 16 f-pairs
KD = D_MODEL // P       # 8 k-tiles (d_model)
KD2 = KD // 2           # 4 k-pairs
TG = SHARD // P         # 4 owned-token tiles
DH = 2                  # d_model halves (512-col matmul free dim)
MAX_CAP = 160           # SR = 8*CAP <= 1280 (SBUF residency bound)
WSCALE = 32.0           # fp8 pre-scale for W1/W2

_prog_cache = {}
_wprep_cache = {}

_bf16 = ml_dtypes.bfloat16
_e4 = ml_dtypes.float8_e4m3
_e5 = ml_dtypes.float8_e5m2


def _chunks(n, step=512):
    out = []
    o = 0
    while o < n:
        L = min(step, n - o)
        out.append((o, L))
        o += L
    return out


def _build_program(CAP):
    import concourse.tile as tile
    from concourse import bacc, mybir, library_config

    f32 = mybir.dt.float32
    bf16 = mybir.dt.bfloat16
    fp8h = mybir.dt.float8e4
    fp8l = mybir.dt.float8e5
    i16 = mybir.dt.int16
    DR = mybir.MatmulPerfMode.DoubleRow
    SR = NCORES * CAP
    G = SR // P
    CH = _chunks(SR)

    nc = bacc.Bacc("TRN2", target_bir_lowering=False, debug=False,
                   num_devices=NCORES)

    xTh = nc.dram_tensor("xTh", [KD2, P, 2, SR], fp8h, kind="ExternalInput").ap()
    xTl = nc.dram_tensor("xTl", [KD2, P, 2, SR], fp8l, kind="ExternalInput").ap()
    W1h = nc.dram_tensor("W1h", [FD, P, KD2 * 2 * P], fp8h,
                         kind="ExternalInput").ap()
    W1l = nc.dram_tensor("W1l", [FD, P, KD2 * 2 * P], fp8l,
                         kind="ExternalInput").ap()
    W2h = nc.dram_tensor("W2h", [FD2, P, 2, D_MODEL], fp8h,
                         kind="ExternalInput").ap()
    W2l = nc.dram_tensor("W2l", [FD2, P, 2, D_MODEL], fp8l,
                         kind="ExternalInput").ap()
    acts = nc.dram_tensor("acts", [P, 2], f32, kind="ExternalInput").ap()
    b1g = nc.dram_tensor("b1g", [P, FD], f32, kind="ExternalInput").ap()
    b1s = nc.dram_tensor("b1s", [P, FD], f32, kind="ExternalInput").ap()
    b1a = nc.dram_tensor("b1a", [P, FD], f32, kind="ExternalInput").ap()
    b2bc = nc.dram_tensor("b2bc", [P, D_MODEL], f32, kind="ExternalInput").ap()
    wct = nc.dram_tensor("wct", [P, G], f32, kind="ExternalInput").ap()
    idx1 = nc.dram_tensor("idx1", [P, SHARD // 16], i16, kind="ExternalInput").ap()
    idx2 = nc.dram_tensor("idx2", [P, SHARD // 16], i16, kind="ExternalInput").ap()
    y_shard = nc.dram_tensor("y_shard", [SHARD, D_MODEL], bf16,
                             kind="ExternalOutput").ap()

    send_h = [nc.dram_tensor(f"send_h{dh}", [SR, 512], bf16).ap()
              for dh in range(DH)]
    recv_h = [nc.dram_tensor(f"recv_h{dh}", [SR, 512], bf16).ap()
              for dh in range(DH)]

    with tile.TileContext(nc) as tc:
        with (
            tc.tile_pool(name="xtp", bufs=1) as xtp,
            tc.tile_pool(name="atp", bufs=1) as atp,
            tc.tile_pool(name="w1p", bufs=3) as w1p,
            tc.tile_pool(name="w2p", bufs=1) as w2p,
            tc.tile_pool(name="smalls", bufs=1) as smalls,
            tc.tile_pool(name="actp", bufs=6) as actp,
            tc.tile_pool(name="sndp", bufs=4) as sndp,
            tc.tile_pool(name="cmb", bufs=1) as cmb,
            tc.tile_pool(name="psm1", bufs=4, space="PSUM") as psm1,
            tc.tile_pool(name="psm2", bufs=4, space="PSUM") as psm2,
        ):
            nc.gpsimd.load_library(library_config.mlp)

            actt = smalls.tile([P, 2], f32, tag="actt")
            nc.sync.dma_start(out=actt[:], in_=acts[:, :])
            b1gt = smalls.tile([P, FD], f32, tag="b1gt")
            nc.sync.dma_start(out=b1gt[:], in_=b1g[:, :])
            b1st = smalls.tile([P, FD], f32, tag="b1st")
            nc.sync.dma_start(out=b1st[:], in_=b1s[:, :])
            b1at = smalls.tile([P, FD], f32, tag="b1at")
            nc.sync.dma_start(out=b1at[:], in_=b1a[:, :])
            b2t = smalls.tile([P, D_MODEL], f32, tag="b2t")
            nc.sync.dma_start(out=b2t[:], in_=b2bc[:, :])
            wctt = smalls.tile([P, G], f32, tag="wctt")
            nc.sync.dma_start(out=wctt[:], in_=wct[:, :])
            ix1 = smalls.tile([P, SHARD // 16], i16, tag="ix1")
            nc.sync.dma_start(out=ix1[:], in_=idx1[:, :])
            ix2 = smalls.tile([P, SHARD // 16], i16, tag="ix2")
            nc.sync.dma_start(out=ix2[:], in_=idx2[:, :])

            xth, xtl = [], []
            for k2 in range(KD2):
                xh = xtp.tile([P, 2, SR], fp8h, tag=f"xh{k2}", name=f"xh{k2}")
                nc.sync.dma_start(out=xh[:], in_=xTh[k2])
                xth.append(xh)
                xl = xtp.tile([P, 2, SR], fp8l, tag=f"xl{k2}", name=f"xl{k2}")
                nc.sync.dma_start(out=xl[:], in_=xTl[k2])
                xtl.append(xl)

            ah8, al8 = [], []
            for p2 in range(FD2):
                ah8.append(atp.tile([P, 2, SR], fp8h, tag=f"ah{p2}",
                                    name=f"ah{p2}"))
                al8.append(atp.tile([P, 2, SR], fp8l, tag=f"al{p2}",
                                    name=f"al{p2}"))

            # ---- mm1 (3-stream fp8 DR) + activation -> ah/al
            for f in range(FD):
                w1fh = w1p.tile([P, KD2, 2, P], fp8h, tag="w1fh")
                nc.sync.dma_start(out=w1fh[:], in_=W1h[f].rearrange(
                    "p (a b q) -> p a b q", a=KD2, b=2))
                w1fl = w1p.tile([P, KD2, 2, P], fp8l, tag="w1fl")
                nc.sync.dma_start(out=w1fl[:], in_=W1l[f].rearrange(
                    "p (a b q) -> p a b q", a=KD2, b=2))
                p2, j = f // 2, f % 2
                for ci, (o, L) in enumerate(CH):
                    ps = psm1.tile([P, 512], mybir.dt.float32, tag="ps1")
                    n3 = 3 * KD2
                    i = 0
                    for (wt, xt) in ((w1fh, xth), (w1fh, xtl), (w1fl, xth)):
                        for k2 in range(KD2):
                            nc.tensor.matmul(ps[:, :L],
                                             lhsT=wt[:, k2, :, :],
                                             rhs=xt[k2][:, :, o:o + L],
                                             start=(i == 0), stop=(i == n3 - 1),
                                             perf_mode=DR)
                            i += 1
                    t1 = actp.tile([P, 512], bf16, tag="t1")
                    t2 = actp.tile([P, 512], bf16, tag="t2")
                    nc.scalar.activation(
                        t1[:, :L], ps[:, :L],
                        mybir.ActivationFunctionType.Erf,
                        bias=b1gt[:, f:f + 1], scale=actt[:, 0:1])
                    nc.scalar.activation(
                        t2[:, :L], ps[:, :L],
                        mybir.ActivationFunctionType.Sigmoid,
                        bias=b1st[:, f:f + 1], scale=actt[:, 1:2])
                    # v = 0.5*t1 + t2 ; temp = (ps + 32*b1) * v = 32*aT
                    nc.vector.scalar_tensor_tensor(
                        out=t1[:, :L], in0=t1[:, :L], scalar=0.5,
                        in1=t2[:, :L],
                        op0=mybir.AluOpType.mult, op1=mybir.AluOpType.add)
                    # temp = (ps + 32*b1)*v, then hi/lo split. Alternate the
                    # op placement between DVE and Pool per tile so both
                    # engines pace at ~half the consumer load (v stays on
                    # DVE; the chain temp -> ah -> al pipelines across
                    # tiles).
                    tmp = actp.tile([P, 512], bf16, tag="tmp")
                    ahs = ah8[p2][:, j, o:o + L]
                    als = al8[p2][:, j, o:o + L]
                    nc.vector.scalar_tensor_tensor(
                        out=tmp[:, :L], in0=ps[:, :L],
                        scalar=b1at[:, f:f + 1], in1=t1[:, :L],
                        op0=mybir.AluOpType.add, op1=mybir.AluOpType.mult)
                    if (f + ci) % 2 == 0:
                        nc.gpsimd.tensor_copy(ahs, tmp[:, :L])
                        nc.vector.tensor_sub(als, tmp[:, :L], ahs)
                    else:
                        nc.vector.tensor_copy(ahs, tmp[:, :L])
                        nc.gpsimd.tensor_sub(als, tmp[:, :L], ahs)

            # ---- mm2 (3-stream fp8 DR, token-major) -> send halves;
            # the dh=0 collective + combine overlap the dh=1 matmuls.
            for dh in range(DH):
                w2th, w2tl = [], []
                for p2 in range(FD2):
                    wh = w2p.tile([P, 2, 512], fp8h, tag=f"w2h_{p2}",
                                  name=f"w2h_{p2}_{dh}")
                    nc.sync.dma_start(
                        out=wh[:], in_=W2h[p2][:, :, dh * 512:(dh + 1) * 512])
                    w2th.append(wh)
                    wl = w2p.tile([P, 2, 512], fp8l, tag=f"w2l_{p2}",
                                  name=f"w2l_{p2}_{dh}")
                    nc.sync.dma_start(
                        out=wl[:], in_=W2l[p2][:, :, dh * 512:(dh + 1) * 512])
                    w2tl.append(wl)
                for g in range(G):
                    ps = psm2.tile([P, 512], mybir.dt.float32, tag="ps2")
                    n3 = 3 * FD2
                    i = 0
                    for (at, wt) in ((ah8, w2th), (al8, w2th), (ah8, w2tl)):
                        for p2 in range(FD2):
                            nc.tensor.matmul(
                                ps[:],
                                lhsT=at[p2][:, :, g * P:(g + 1) * P],
                                rhs=wt[p2][:, :, :],
                                start=(i == 0), stop=(i == n3 - 1),
                                perf_mode=DR)
                            i += 1
                    snd = sndp.tile([P, 512], bf16, tag="snd")
                    nc.vector.tensor_scalar_mul(snd[:], ps[:], wctt[:, g:g + 1])
                    nc.sync.dma_start(out=send_h[dh][g * P:(g + 1) * P, :],
                                      in_=snd[:])

                nc.gpsimd.collective_compute(
                    "AllToAll",
                    mybir.AluOpType.bypass,
                    replica_groups=[list(range(NCORES))],
                    ins=[send_h[dh][:, :]],
                    outs=[recv_h[dh][:, :]],
                )

                # combine: gather each owned token's two contribution rows
                g1 = cmb.tile([P, TG, 512], bf16, tag=f"g1_{dh}", name=f"g1_{dh}")
                nc.gpsimd.dma_gather(
                    out_ap=g1[:, :, :], in_ap=recv_h[dh][:, :], idxs_ap=ix1[:],
                    num_idxs=SHARD, num_idxs_reg=SHARD, elem_size=512)
                g2 = cmb.tile([P, TG, 512], bf16, tag=f"g2_{dh}", name=f"g2_{dh}")
                nc.gpsimd.dma_gather(
                    out_ap=g2[:, :, :], in_ap=recv_h[dh][:, :], idxs_ap=ix2[:],
                    num_idxs=SHARD, num_idxs_reg=SHARD, elem_size=512)
                nc.vector.tensor_add(g1[:, :, :], g1[:, :, :], g2[:, :, :])
                for tg in range(TG):
                    nc.vector.tensor_add(g1[:, tg, :], g1[:, tg, :],
                                         b2t[:, dh * 512:(dh + 1) * 512])
                    nc.sync.dma_start(
                        out=y_shard[tg * P:(tg + 1) * P,
                                    dh * 512:(dh + 1) * 512],
                        in_=g1[:, tg, :])

    nc.compile()
    return nc


def _route(x_flat, Wg, bg):
    logits = x_flat.astype(np.float32) @ Wg.astype(np.float32) + bg
    order = np.argsort(-logits, axis=1, kind="stable")
    i1, i2 = order[:, 0], order[:, 1]
    s1 = np.take_along_axis(logits, i1[:, None], 1)[:, 0]
    s2 = np.take_along_axis(logits, i2[:, None], 1)[:, 0]
    e = np.exp((s2 - s1).astype(np.float32))
    w1 = 1.0 / (1.0 + e)
    w2 = e * w1
    return i1, i2, w1.astype(np.float32), w2.astype(np.float32)


def _hi_lo(a):
    hi = np.clip(a, -240, 240).astype(_e4)
    lo = (a - hi.astype(np.float32)).astype(_e5)
    return hi, lo


def _prep_weights(W1, W2):
    key = (id(W1), id(W2))
    hit = _wprep_cache.get(key)
    if hit is not None:
        return hit
    W1s = np.asarray(W1, np.float32) * WSCALE
    # [e, f, p, k2, j, q] = W1s[e, (2*k2+j)*128+p, f*128+q]
    W1r = np.ascontiguousarray(
        W1s.reshape(N_EXPERTS, KD2, 2, P, FD, P).transpose(0, 4, 3, 1, 2, 5)
        .reshape(N_EXPERTS, FD, P, KD2 * 2 * P))
    W1rh, W1rl = _hi_lo(W1r)
    W2s = np.asarray(W2, np.float32) * WSCALE
    # [e, p2, p, j, d] = W2s[e, (2*p2+j)*128+p, d]
    W2r = np.ascontiguousarray(
        W2s.reshape(N_EXPERTS, FD2, 2, P, D_MODEL).transpose(0, 1, 3, 2, 4))
    W2rh, W2rl = _hi_lo(W2r)
    _wprep_cache.clear()
    _wprep_cache[key] = (W1rh, W1rl, W2rh, W2rl)
    return _wprep_cache[key]


def _prepare(x, W1, b1, W2, b2, Wg, bg):
    x = np.asarray(x, np.float32)
    b1 = np.asarray(b1, np.float32)
    b2 = np.asarray(b2, np.float32)
    x_flat = np.ascontiguousarray(x.reshape(T, D_MODEL))
    i1, i2, w1, w2 = _route(x_flat, np.asarray(Wg, np.float32),
                            np.asarray(bg, np.float32))
    Wq = _prep_weights(W1, W2)

    jobs = {}  # expert -> (ids ascending = sorted by owner, wts)
    for e in range(N_EXPERTS):
        sel = (i1 == e) | (i2 == e)
        ids = np.nonzero(sel)[0]
        wts = np.where(i1[ids] == e, w1[ids], w2[ids]).astype(np.float32)
        jobs[e] = (ids, wts)
    return x_flat, jobs, (Wq, b1, b2)


def _wrap_idx(r):
    """[n] int -> [128, n/16] int16 (wrapped by 16, replicated 8x)."""
    n = len(r)
    w = np.zeros((16, n // 16), np.int16)
    w[np.arange(n) % 16, np.arange(n) // 16] = r
    return np.tile(w, (8, 1))


def _pass_maps(x_flat, jobs, consts, first_pass=True, strict=False):
    (W1rh, W1rl, W2rh, W2rl), b1, b2 = consts

    bucket_count = np.zeros((NCORES, NCORES), np.int64)
    for e in range(NCORES):
        ids, _ = jobs[e]
        own = ids // SHARD
        for o in range(NCORES):
            bucket_count[e, o] += (own == o).sum()
    CAP = max(16, int(-(-bucket_count.max() // 16) * 16))
    assert CAP <= MAX_CAP
    SR = NCORES * CAP
    G = SR // P

    # recv row (on the owner) of each token contribution
    src_rows = np.full((T, 2), -1, np.int64)
    slot_of = {}
    for e in range(NCORES):
        ids, _ = jobs[e]
        own = ids // SHARD
        ks = np.empty(len(ids), np.int64)
        fill = np.zeros(NCORES, np.int64)
        for o in range(NCORES):
            m = own == o
            n = int(m.sum())
            ks[m] = fill[o] + np.arange(n)
            fill[o] += n
        slot_of[e] = own * CAP + ks
        rows_recv = e * CAP + ks
        which = (src_rows[ids, 0] >= 0).astype(np.int64)
        src_rows[ids, which] = rows_recv
    if strict:
        assert (src_rows >= 0).all()

    sq2 = np.float32(1.0 / np.sqrt(2.0))
    in_maps = []
    for c in range(NCORES):
        e = c
        ids, wts = jobs[e]
        xTc = np.zeros((D_MODEL, SR), np.float32)
        wcol = np.zeros(SR, np.float32)
        if len(ids):
            slots = slot_of[e]
            xTc[:, slots] = x_flat[ids].T
            wcol[slots] = wts / np.float32(WSCALE * WSCALE)
        xh, xl = _hi_lo(xTc)
        # [k2, p, j, n] = x[(2*k2+j)*128+p, n]
        xh = np.ascontiguousarray(
            xh.reshape(KD2, 2, P, SR).transpose(0, 2, 1, 3))
        xl = np.ascontiguousarray(
            xl.reshape(KD2, 2, P, SR).transpose(0, 2, 1, 3))
        even = (e % 2 == 0)
        b1_cols = np.ascontiguousarray(b1[e].reshape(FD, P).T)  # [P, FD]
        actsel = np.zeros((P, 2), np.float32)
        actsel[:, 0] = sq2 / WSCALE if even else 0.0
        actsel[:, 1] = 0.0 if even else 1.0 / WSCALE
        b1gv = b1_cols * sq2 if even else np.zeros((P, FD), np.float32)
        b1sv = np.zeros((P, FD), np.float32) if even else b1_cols
        b2v = b2[e] if first_pass else np.zeros(D_MODEL, np.float32)
        tok0 = c * SHARD
        r1 = src_rows[tok0:tok0 + SHARD, 0]
        r2 = src_rows[tok0:tok0 + SHARD, 1]
        if not strict:
            # under npass splitting a token's two contributions may land in
            # different passes; point the missing one at a universal pad row
            # (zero on every core).
            pad_slot = _find_pad_row(bucket_count, CAP)
            r1 = np.where(r1 < 0, pad_slot, r1)
            r2 = np.where(r2 < 0, pad_slot, r2)
        in_maps.append({
            "xTh": xh, "xTl": xl,
            "W1h": W1rh[e], "W1l": W1rl[e],
            "W2h": W2rh[e], "W2l": W2rl[e],
            "acts": actsel, "b1g": np.ascontiguousarray(b1gv),
            "b1s": np.ascontiguousarray(b1sv),
            "b1a": np.ascontiguousarray(b1_cols * np.float32(WSCALE)),
            "b2bc": np.ascontiguousarray(
                np.broadcast_to(b2v, (P, D_MODEL)).astype(np.float32)),
            "wct": np.ascontiguousarray(wcol.reshape(G, P).T),
            "idx1": _wrap_idx(r1), "idx2": _wrap_idx(r2),
        })
    return (CAP,), in_maps


def _find_pad_row(bucket_count, CAP):
    """Recv row index that is a zero pad slot on every core: pick (e, k)
    with k >= max_o bucket_count[e, o]."""
    per_e_max = bucket_count.max(axis=1)
    e = int(per_e_max.argmin())
    k = int(per_e_max[e])
    assert k < CAP, "no universal pad slot (all buckets full)"
    return e * CAP + k


def make_in_maps(x, W1, b1, W2, b2, Wg, bg):
    x_flat, jobs, consts = _prepare(x, W1, b1, W2, b2, Wg, bg)
    return _pass_maps(x_flat, jobs, consts, strict=True)


def get_program(key):
    if key not in _prog_cache:
        _prog_cache[key] = _build_program(*key)
    return _prog_cache[key]


def kernel(x, W1, b1, W2, b2, Wg, bg):
    from concourse.bass_utils import run_bass_kernel_spmd

    x_flat, jobs, consts = _prepare(x, W1, b1, W2, b2, Wg, bg)
    maxbucket = 0
    for e in range(N_EXPERTS):
        own = jobs[e][0] // SHARD
        if len(own):
            maxbucket = max(maxbucket, int(np.bincount(own).max()))
    npass = max(1, -(-maxbucket // MAX_CAP))
    out = None
    for p in range(npass):
        jobs_p = {e: (ids[p::npass], wts[p::npass])
                  for e, (ids, wts) in jobs.items()}
        key, in_maps = _pass_maps(x_flat, jobs_p, consts,
                                  first_pass=(p == 0), strict=(npass == 1))
        nc = get_program(key)
        res = run_bass_kernel_spmd(nc, in_maps, list(range(NCORES)))
        full = np.concatenate(
            [res.results[c]["y_shard"].astype(np.float32)
             for c in range(NCORES)], axis=0)
        out = full if out is None else out + full
    return np.ascontiguousarray(out.reshape(B, S, D_MODEL))
